# revision 47
# baseline (speedup 1.0000x reference)
"""Multi-head attention (B=8, S=1500, E=1024, H=16, D=64) on 8 trn2 NeuronCores.

Sharding: pure data-parallel over batch — core b computes batch element b
end-to-end (no collectives). Host pre-transposes x and the weights so every
device-side matmul has its contraction dim on the SBUF partition axis, and
folds the 1/sqrt(D) scale into Wq/bq and the V-bias into the output bias
(bo_eff = bo + Wo @ bv), so the device kernel never touches bv.

Device pipeline per core (bf16 matmul operands, f32 psum):
  QT = (Wq/8)^T-proj of x^T   [1024, 1500]  (f-on-partition; bias bq/8 per-partition)
  KT = Wk^T-proj              [1024, 1500]
  V_aug = x @ Wv^T with a ones-column appended per head  [1500, 16*65]
  per (i-chunk, head-pair): scoresT[j, i] via matmul(lhsT=KT_h, rhs=QT_h),
    the two heads' D=64 contractions row-tiled at PE offsets (0,0)/(64,0)
    so they stream concurrently;
    exp on ACT (both halves in one ACTIVATE);
    causal masking on diagonal blocks via a post-exp gpsimd
    affine_select (zero-fill above the diagonal, both halves one op);
  out^T + rowsums in ONE matmul: lhsT=[V_h | 1] (65 cols), rhs=attnT;
  normalize: sums row -> bf16, rank-1 matmul broadcast across 64
    partitions, fast-reciprocal, multiply on eviction -> AO^T;
  yT = Wo^T-proj of AO^T + bo_eff -> DRAM [1024, 1500] bf16 (host casts
    to f32 and transposes back).
Scheduling: ft=0 q/k weight slices interleaved with x-tile DMAs so the
first projection chains start as x lands; wo loads deferred past the
ft=1 prefetch; next-ft projections pumped into the attention stretches
as 4-matmul half-chains every 2 j-blocks (the exp on ACT is the local
bottleneck there); psum-release-gating norm chain at high priority;
eviction copies on nc.any so the scheduler balances ACT/DVE.
"""

import sys
import numpy as np
import ml_dtypes

for _p in ("/opt/trn_rl_repo",):
    if _p not in sys.path:
        sys.path.append(_p)

import concourse.bass as bass
import concourse.mybir as mybir
import concourse.tile as tile
from concourse import bacc
from concourse.bass_utils import run_bass_kernel_spmd

F32 = mybir.dt.float32

B, S, E, H, D = 8, 1500, 1024, 16, 64
P = 128
NEG = -1e9


def _chunks(total, step):
    return [(c0, min(step, total - c0)) for c0 in range(0, total, step)]


def _wslices(dram_ap, col0, cols):
    """[E, E] weight -> [P, E//P, cols] AP for a column slice (k on partition)."""
    return dram_ap.rearrange("(kt p) f -> p kt f", p=P)[:, :, col0:col0 + cols]


def build(causal: bool, mm_dt=mybir.dt.bfloat16):
    KT_N = E // P            # k-tiles over the embedding dim
    FT_N = E // P            # f-tiles
    R_CH = _chunks(S, 512)   # i/r chunks
    JB = _chunks(S, P)       # j blocks
    H_LOC = E // D
    nc = bacc.Bacc("TRN2", target_bir_lowering=False, debug=False, num_devices=8)
    MD = mm_dt  # dtype for every matmul operand chain
    NRM = mybir.dt.float32r if MD == mybir.dt.bfloat16 else MD

    xT = nc.dram_tensor("xT", [E, S], MD, kind="ExternalInput")
    wqT = nc.dram_tensor("wqT", [E, E], MD, kind="ExternalInput")
    wkT = nc.dram_tensor("wkT", [E, E], MD, kind="ExternalInput")
    wvT = nc.dram_tensor("wvT", [E, E], MD, kind="ExternalInput")
    woT = nc.dram_tensor("woT", [E, E], MD, kind="ExternalInput")
    bq = nc.dram_tensor("bq", [E], F32, kind="ExternalInput")
    bo = nc.dram_tensor("bo", [E], F32, kind="ExternalInput")
    maskT = None
    if not causal:
        maskT = nc.dram_tensor("maskT", [S, S], F32, kind="ExternalInput")
    yT = nc.dram_tensor("yT", [E, S], MD, kind="ExternalOutput")

    def mm(ap):
        return ap

    nc._allow_low_precision_reason = "low-precision matmul operand chain"
    with tile.TileContext(nc) as tc:
        with (
            tc.tile_pool(name="persist", bufs=1) as pers,
            tc.tile_pool(name="wqkp", bufs=2) as wqkp,
            tc.tile_pool(name="wvp", bufs=1) as wvp,
            tc.tile_pool(name="wop", bufs=1) as wop,
            tc.tile_pool(name="attn", bufs=5) as apool,
            tc.tile_pool(name="small", bufs=3) as spool,
            tc.tile_pool(name="evp", bufs=3) as evp,
            tc.tile_pool(name="psP", bufs=1, space="PSUM") as psP,
            tc.tile_pool(name="psS", bufs=1, space="PSUM") as psS,
            tc.tile_pool(name="psO", bufs=1, space="PSUM") as psO,
            tc.tile_pool(name="psY", bufs=1, space="PSUM") as psY,
        ):
            ones64 = pers.tile([1, D], MD, name="ones64")
            if MD == mybir.dt.float32r:
                nc.vector.memset(ones64[:].bitcast(F32), 1.0)
            else:
                nc.vector.memset(ones64[:], 1.0)
            bq_sb = pers.tile([P, FT_N], F32, name="bq_sb")
            nc.sync.dma_start(out=bq_sb[:], in_=bq.ap().rearrange("(t p) -> p t", p=P))
            bo_sb = pers.tile([P, FT_N], F32, name="bo_sb")
            nc.sync.dma_start(out=bo_sb[:], in_=bo.ap().rearrange("(t p) -> p t", p=P))

            # PE warm-up: the HAM clock gate holds the PE at 1.2 GHz until
            # ~3.4us of sustained matmul activity. The first ~10us are
            # DMA-bound with no real PE work, so run small dummy matmuls on
            # memset data (no DMA dependency) to un-throttle the clock
            # before the real projection chains start.
            warm = pers.tile([P, 40], MD, name="warm")
            nc.vector.memset(warm[:], 0.5)
            psw = psS.tile([P, 2, 512], F32, name="psw", tag="ps", bufs=2)
            for _ in range(200):
                nc.tensor.matmul(
                    psw[:8, 0, :32], mm(warm[:, 0:8]), mm(warm[:, 8:40]),
                    start=True, stop=True,
                )
            wsink = pers.tile([8, 32], F32, name="wsink")
            nc.vector.tensor_copy(out=wsink[:], in_=psw[:8, 0, :32])



            XT = [pers.tile([P, S], MD, name=f"xt{kt}") for kt in range(KT_N)]
            QT = [pers.tile([P, S], MD, name=f"qt{ft}") for ft in range(FT_N)]
            KTs = [pers.tile([P, S], MD, name=f"kt{ft}") for ft in range(FT_N)]
            VA = [pers.tile([P, H_LOC * (D + 1)], MD, name=f"va{rt}")
                  for rt in range(len(JB))]
            AOT = [pers.tile([P, S], MD, name=f"aot{ft}") for ft in range(FT_N)]

            # ---- V projection (natural layout, ones column appended) ----
            hpc = 512 // D  # heads per 512-wide f chunk
            fchunks = _chunks(E, 512)
            wv_tiles = [wvp.tile([P, KT_N, 512], MD, name=f"wv{fc}", tag=f"wv{fc}")
                        for fc in range(len(fchunks))]

            def load_xt():
                for kt in range(KT_N):
                    nc.sync.dma_start(out=XT[kt][:],
                                      in_=xT[kt * P:(kt + 1) * P, :])

            def load_wv():
                for fc, (f0, fw) in enumerate(fchunks):
                    for kt in range(KT_N):
                        nc.sync.dma_start(
                            out=wv_tiles[fc][:, kt, :fw],
                            in_=_wslices(wvT.ap(), f0, fw)[:, kt, :])

            def emit_v(rts):
                for rt in rts:
                    if rt >= len(JB):
                        continue
                    r0, rsz = JB[rt]
                    for fc, (f0, fw) in enumerate(fchunks):
                        wt = wv_tiles[fc]
                        ps = psP.tile([P, 512], F32, name="pv", tag="pp", bufs=2)
                        for kt in range(KT_N):
                            nc.tensor.matmul(
                                ps[:rsz, :fw],
                                mm(XT[kt][:, r0:r0 + rsz]),
                                mm(wt[:, kt, :fw]),
                                start=(kt == 0), stop=(kt == KT_N - 1),
                            )
                        dst = VA[rt][:].rearrange("p (h c) -> p h c", c=D + 1)
                        nc.any.tensor_copy(
                            out=dst[:rsz, fc * hpc:fc * hpc + fw // D, 0:D],
                            in_=ps[:rsz, :fw].rearrange("p (h d) -> p h d", d=D),
                        )
                    va3 = VA[rt][:].rearrange("p (h c) -> p h c", c=D + 1)
                    if MD == mybir.dt.float32r:
                        nc.gpsimd.memset(va3[:rsz, :, D:D + 1].bitcast(F32), 1.0)
                    else:
                        nc.gpsimd.memset(va3[:rsz, :, D:D + 1], 1.0)

            def load_wqk(ft, with_xt=False):
                """DMA the ft-th q/k weight column slices; with_xt interleaves
                the x-tile loads (x first per kt) so the first projection
                matmuls start as soon as each x tile lands."""
                wts = []
                for wdram in (wqT, wkT):
                    wt = wqkp.tile([P, KT_N, P], MD, name="wqk", tag="wqk",
                                   bufs=4)
                    wts.append(wt)
                if with_xt:
                    for kt in range(KT_N):
                        nc.sync.dma_start(out=XT[kt][:],
                                          in_=xT[kt * P:(kt + 1) * P, :])
                        for wt, wdram in zip(wts, (wqT, wkT)):
                            nc.sync.dma_start(
                                out=wt[:, kt, :],
                                in_=_wslices(wdram.ap(), ft * P, P)[:, kt, :])
                else:
                    for wt, wdram in zip(wts, (wqT, wkT)):
                        nc.sync.dma_start(
                            out=wt[:], in_=_wslices(wdram.ap(), ft * P, P))
                return wts

            def proj_qk_gen(ft, wts=None):
                if wts is None:
                    wts = load_wqk(ft)
                for which, wt, dst in (("q", wts[0], QT), ("k", wts[1], KTs)):
                    for rc, (c0, cw) in enumerate(R_CH):
                        ps = psP.tile([P, 512], F32, name="pp", tag="pp", bufs=2)
                        for kt in range(KT_N):
                            nc.tensor.matmul(
                                ps[:, :cw],
                                mm(wt[:, kt, :]),
                                mm(XT[kt][:, c0:c0 + cw]),
                                start=(kt == 0), stop=(kt == KT_N - 1),
                            )
                            if kt == KT_N // 2 - 1:
                                yield
                        if which == "q":
                            nc.any.tensor_scalar(
                                out=dst[ft][:, c0:c0 + cw], in0=ps[:, :cw],
                                scalar1=bq_sb[:, ft:ft + 1], scalar2=None,
                                op0=mybir.AluOpType.add,
                            )
                        else:
                            nc.any.tensor_copy(
                                out=dst[ft][:, c0:c0 + cw], in_=ps[:, :cw])
                        yield

            def proj_qk(ft, wts=None):
                for _ in proj_qk_gen(ft, wts):
                    pass

            def attn_ft(ic, ft, mtiles, filler=None, filler_every=4):
                c0, cw = R_CH[ic]
                nblk = (min(len(JB), (c0 + cw + P - 1) // P)
                        if causal else len(JB))
                pso = [psO.tile([D + 1, 512], F32, name=f"po{half}",
                                tag="po", bufs=2)
                       for half in range(2)]
                # diagonal-containing blocks first so the chunk-end attnV
                # gates on a short (non-masked) exp chain
                if causal:
                    cut = max(0, nblk - (cw + P - 1) // P)
                    order = list(range(cut, nblk)) + list(range(cut))
                else:
                    order = list(range(nblk))
                for n_i, jb in enumerate(order):
                    j0, jsz = JB[jb]
                    vo = max(0, j0 - c0) if causal else 0
                    diag = causal and j0 >= c0
                    # both halves' scores land in one 2-bank psum pair so a
                    # single ACTIVATE exps them together (halves ACT op count)
                    psp = psS.tile([P, 2, 512], F32, name="psp",
                                   tag="ps", bufs=2)
                    for half in range(2):
                        d0 = D * half
                        nc.tensor.matmul(
                            psp[:jsz, half, vo:cw],
                            mm(KTs[ft][d0:d0 + D, j0:j0 + jsz]),
                            mm(QT[ft][d0:d0 + D, c0 + vo:c0 + cw]),
                            start=True, stop=True,
                            tile_position=(d0, 0),
                        )
                    if not causal:
                        for half in range(2):
                            nc.vector.tensor_tensor(
                                out=psp[:jsz, half, :cw],
                                in0=psp[:jsz, half, :cw],
                                in1=mtiles[jb][:jsz, :cw],
                                op=mybir.AluOpType.add,
                            )
                    atp = apool.tile([P, 2, 512], MD, name="atp")
                    nc.scalar.activation(
                        out=atp[:jsz, :, vo:cw], in_=psp[:jsz, :, vo:cw],
                        func=mybir.ActivationFunctionType.Exp,
                    )
                    if diag:
                        # zero attn where j' > t' on the diagonal square —
                        # on GPSIMD (near-idle) instead of PE mask-matmuls
                        # or DVE multiplies; both halves in one op.
                        nc.gpsimd.affine_select(
                            out=atp[:jsz, :, vo:vo + jsz],
                            in_=atp[:jsz, :, vo:vo + jsz],
                            pattern=[[0, 2], [1, jsz]],
                            compare_op=mybir.AluOpType.is_ge,
                            fill=0.0, base=0, channel_multiplier=-1,
                        )
                    va3 = VA[jb][:].rearrange("p (h c) -> p h c", c=D + 1)
                    for half in range(2):
                        nc.tensor.matmul(
                            pso[half][:, vo:cw],
                            mm(va3[:jsz, 2 * ft + half, :]),
                            mm(atp[:jsz, half, vo:cw]),
                            start=(n_i == 0), stop=(n_i == nblk - 1),
                        )
                    if filler is not None and n_i % filler_every == filler_every - 1:
                        filler()
                # sums to bf16 SBUF, rank-1 matmul broadcast across 64
                # partitions, reciprocal of the broadcast, normalize on evict.
                # High priority: this tiny chain gates the pso psum release
                # for the next (ic, ft) chunk's attnV accumulation.
                with tc.high_priority():
                    ssums = []
                    for half in range(2):
                        ssum = spool.tile([1, 512], MD, name=f"ssum{half}",
                                          tag="ssum")
                        nc.vector.tensor_copy(
                            out=ssum[:, :cw], in_=pso[half][D:D + 1, :cw])
                        ssums.append(ssum)
                    for half in range(2):
                        d0 = D * half
                        psb = psP.tile([D, 512], F32, name="psb", tag="pp",
                                       bufs=2)
                        nc.tensor.matmul(
                            psb[:, :cw], mm(ones64[:, :]),
                            mm(ssums[half][:, :cw]),
                            start=True, stop=True,
                        )
                        rb = spool.tile([D, 512], F32, name="rb")
                        nc.vector.reciprocal_approx_fast(
                            out=rb[:, :cw], in_=psb[:, :cw])
                        nc.vector.tensor_tensor(
                            out=AOT[ft][d0:d0 + D, c0:c0 + cw],
                            in0=pso[half][0:D, :cw], in1=rb[:, :cw],
                            op=mybir.AluOpType.mult,
                        )

            def emit_yt(ot, rc, wo_t):
                c0, cw = R_CH[rc]
                psy = psP.tile([P, 512], F32, name="py", tag="pp", bufs=2)
                for ft in range(FT_N):
                    nc.tensor.matmul(
                        psy[:, :cw],
                        mm(wo_t[:, ft, :]),
                        mm(AOT[ft][:, c0:c0 + cw]),
                        start=(ft == 0), stop=(ft == FT_N - 1),
                    )
                yt = evp.tile([P, 512], MD, name="yt", tag="yt")
                nc.any.tensor_scalar(
                    out=yt[:, :cw], in0=psy[:, :cw],
                    scalar1=bo_sb[:, ot:ot + 1], scalar2=None,
                    op0=mybir.AluOpType.add,
                )
                nc.sync.dma_start(
                    out=yT[ot * P:(ot + 1) * P, c0:c0 + cw], in_=yt[:, :cw])

            if causal:
                # startup order: x tiles interleaved with the small ft=0 q/k
                # weight slices so the first projection chains start as each
                # x tile lands, then the 2MB of V weights. wo (needed only at
                # ft=7) is DMA'd later, after the ft=1 weight prefetch.
                wts0 = load_wqk(0, with_xt=True)
                proj_qk(0, wts0)
                load_wv()
                nb0 = min(len(JB), (R_CH[0][0] + R_CH[0][1] + P - 1) // P)
                emit_v(range(nb0))
                wo_tiles = [wop.tile([P, KT_N, P], MD, name=f"wo{ot}",
                                     tag=f"wo{ot}")
                            for ot in range(FT_N)]
                nbp = nb0
                for ft in range(FT_N):
                    if ft == 1:
                        for ot in range(FT_N):
                            nc.sync.dma_start(
                                out=wo_tiles[ot][:],
                                in_=_wslices(woT.ap(), ot * P, P))
                    gen = proj_qk_gen(ft + 1) if ft + 1 < FT_N else None
                    ytg = None

                    def pump():
                        if gen is not None:
                            next(gen, None)

                    def pump_yt():
                        if ytg is not None:
                            next(ytg, None)

                    def yt_gen_for(rc):
                        for ot in range(FT_N):
                            emit_yt(ot, rc, wo_tiles[ot])
                            yield

                    for ic in range(len(R_CH)):
                        if ft == FT_N - 1:
                            # last ft has no next-ft projection filler: use
                            # the previous ic's now-ready yT chunk instead
                            attn_ft(ic, ft, None, filler=pump_yt,
                                    filler_every=2)
                            if ytg is not None:
                                for _ in ytg:
                                    pass
                            ytg = yt_gen_for(ic)
                        else:
                            attn_ft(ic, ft, None, filler=pump,
                                    filler_every=2)
                        if ft == 0 and ic + 1 < len(R_CH):
                            c0n, cwn = R_CH[ic + 1]
                            nbn = min(len(JB), (c0n + cwn + P - 1) // P)
                            emit_v(range(nbp, nbn))
                            nbp = nbn
                    if gen is not None:
                        for _ in gen:
                            pass
                    if ytg is not None:
                        for _ in ytg:
                            pass
            else:
                load_xt()
                load_wv()
                emit_v(range(len(JB)))
                for ft in range(FT_N):
                    proj_qk(ft)
                with tc.tile_pool(name="maskp", bufs=1) as mpool:
                    for ic, (c0, cw) in enumerate(R_CH):
                        mtiles = []
                        for jb, (j0, jsz) in enumerate(JB):
                            mt = mpool.tile([P, 512], F32, name=f"m{jb}")
                            nc.sync.dma_start(
                                out=mt[:jsz, :cw],
                                in_=maskT[j0:j0 + jsz, c0:c0 + cw])
                            mtiles.append(mt)
                        for ft in range(FT_N):
                            attn_ft(ic, ft, mtiles)
                for ot in range(FT_N):
                    wt = wop.tile([P, KT_N, P], MD, name=f"wo{ot}", tag="wo",
                                  bufs=2)
                    nc.sync.dma_start(out=wt[:], in_=_wslices(woT.ap(), ot * P, P))
                    for rc in range(len(R_CH)):
                        emit_yt(ot, rc, wt)

    nc.compile()
    return nc


_CACHE: dict = {}


def _get_nc(causal: bool):
    if causal not in _CACHE:
        _CACHE[causal] = build(causal)
    return _CACHE[causal]


def _is_causal(mask: np.ndarray) -> bool:
    if mask.shape != (S, S):
        return False
    expect = np.where(np.tril(np.ones((S, S), dtype=bool)), np.float32(0.0),
                      np.float32(NEG))
    return bool(np.array_equal(mask, expect))


MM_NP = ml_dtypes.bfloat16  # numpy dtype matching build()'s default mm_dt


def prep_inputs(x, mask, Wq, bq, Wk, Wv, bv, Wo, bo):
    """Host-side preprocessing shared by kernel() and the bench harness."""
    scale = np.float32(1.0 / np.sqrt(D))
    xT = np.ascontiguousarray(np.transpose(x, (0, 2, 1)).astype(np.float32)).astype(MM_NP)
    common = {
        "wqT": np.ascontiguousarray((Wq.astype(np.float32) * scale).T).astype(MM_NP),
        "wkT": np.ascontiguousarray(Wk.astype(np.float32).T).astype(MM_NP),
        "wvT": np.ascontiguousarray(Wv.astype(np.float32).T).astype(MM_NP),
        "woT": np.ascontiguousarray(Wo.astype(np.float32).T).astype(MM_NP),
        "bq": (bq.astype(np.float32) * scale),
        "bo": (bo.astype(np.float32) + Wo.astype(np.float32) @ bv.astype(np.float32)),
    }
    causal = _is_causal(np.asarray(mask))
    if not causal:
        common["maskT"] = np.ascontiguousarray(np.asarray(mask, np.float32).T)
    in_maps = [dict(common, xT=xT[b]) for b in range(B)]
    return causal, in_maps


_RUNNER: dict = {}


def _get_runner(causal: bool):
    """Compile once per mask-variant; cache the jitted SPMD executable."""
    if causal in _RUNNER:
        return _RUNNER[causal]
    import jax
    from jax.sharding import Mesh, PartitionSpec, NamedSharding
    import warnings
    with warnings.catch_warnings():
        warnings.simplefilter("ignore")
        from jax.experimental.shard_map import shard_map
    from concourse import bass2jax
    from concourse.bass2jax import _bass_exec_p, install_neuronx_cc_hook

    nc = _get_nc(causal)
    install_neuronx_cc_hook()
    partition_name = (nc.partition_id_tensor.name
                      if nc.partition_id_tensor else None)
    in_names, out_names, out_avals = [], [], []
    for alloc in nc.m.functions[0].allocations:
        if not isinstance(alloc, mybir.MemoryLocationSet):
            continue
        name = alloc.memorylocations[0].name
        if alloc.kind == "ExternalInput":
            if name != partition_name:
                in_names.append(name)
        elif alloc.kind == "ExternalOutput":
            out_names.append(name)
            out_avals.append(jax.core.ShapedArray(
                tuple(alloc.tensor_shape), mybir.dt.np(alloc.dtype)))
    n_params = len(in_names)
    n_outs = len(out_names)

    def _body(*args):
        operands = list(args)
        names = list(in_names) + list(out_names)
        if partition_name is not None:
            operands.append(bass2jax.partition_id_tensor())
            names.append(partition_name)
        outs = _bass_exec_p.bind(
            *operands,
            out_avals=tuple(out_avals),
            in_names=tuple(names),
            out_names=tuple(out_names),
            lowering_input_output_aliases=(),
            sim_require_finite=True,
            sim_require_nnan=True,
            nc=nc,
        )
        return tuple(outs)

    devices = jax.devices()[:B]
    mesh = Mesh(np.asarray(devices), ("core",))
    in_specs = (PartitionSpec("core"),) * (n_params + n_outs)
    out_specs = (PartitionSpec("core"),) * n_outs
    fn = jax.jit(
        shard_map(_body, mesh=mesh, in_specs=in_specs, out_specs=out_specs,
                  check_rep=False),
        donate_argnums=tuple(range(n_params, n_params + n_outs)),
        keep_unused=True,
    )
    runner = (fn, in_names, out_names, out_avals)
    _RUNNER[causal] = runner
    return runner


def kernel(x, mask, Wq, bq, Wk, Wv, bv, Wo, bo):
    causal, in_maps = prep_inputs(x, mask, Wq, bq, Wk, Wv, bv, Wo, bo)
    fn, in_names, out_names, out_avals = _get_runner(causal)
    cat = [np.concatenate([np.asarray(m[n]) for m in in_maps], axis=0)
           for n in in_names]
    zs = [np.zeros((B * a.shape[0], *a.shape[1:]), a.dtype) for a in out_avals]
    outs = fn(*cat, *zs)
    yT = np.asarray(outs[out_names.index("yT")]).reshape(B, E, S)
    out = np.ascontiguousarray(yT.transpose(0, 2, 1).astype(np.float32))
    return out

